# revision 11
# baseline (speedup 1.0000x reference)
"""ConvKB-style GNN scoring kernel for Trainium2 (8 NeuronCores).

Math (per triple (h, r, t)):
    ph = ent_emb[h] @ rel_W[r]          # (D,)
    pt = ent_emb[t] @ rel_W[r]          # (D,)
    z[c,d]   = cw0_c*ph[d] + cw1_c*re_r[d] + cw2_c*pt[d] + cb_c
    out      = sum_{c,d} fc[c,d] * relu(z[c,d]) + fc_b

Design (v2):
  - Relations are rank-batched across the 8 cores (batch j = relations ranked
    8j..8j+7 by triple count, one per core) so every core has an identical
    column layout (SPMD) with minimal padding (cols ~ 2100 per core).
  - Host gathers head/tail entity rows into an fp16 (D, 2*cols) tile and the
    per-relation rel_emb broadcast into an fp16 (D, cols) tile.
  - Projection: per relation group one (D,D)x(D,2cap) fp16 matmul -> PSUM,
    then copy/convert to fp16 SBUF ph/pt tiles.
  - Conv+relu+fc per channel c (all FULL-WIDTH ops, ~150 big instrs instead of
    ~1500 tiny per-(c,group) ones):
      q = (oth * ratio_c) + main                [stst, DVE]
      v = (reb * sgn*wr_c) +/- q                [stst, DVE or GPSIMD]
      r = max(v + sgn*cbr_c, 0)                 [tensor_scalar 4x DVE / ACT]
      po += fcs_c^T @ r                         [PE, PSUM-accumulated]
    where z_c = s_c*(main + ratio_c*oth + wr_c*reb + cbr_c), s_c folded into
    the fc weights, sign handled via subtract + negated scalars.
  - Everything on-chip is fp16 (DVE 2x on stst, 4x on relu; halved DMA).
"""

import sys
from contextlib import ExitStack

import numpy as np

sys.path.insert(0, "/opt/trn_rl_repo")

import concourse.bass as bass
import concourse.bacc as bacc
import concourse.tile as tile
from concourse import mybir
from concourse.bass_utils import run_bass_kernel_spmd

N_CORES = 8
R_TOT = 237
D = 128
C = 50
PCHUNK = 512  # max PSUM bank columns (f32) for the fc accumulation

F32 = mybir.dt.float32
F16 = mybir.dt.float16

# engine assignment knobs (tuned from trace).  GPSIMD cannot run
# scalar_tensor_tensor (Pool-engine ISA check rejects it), so both stst
# passes stay on DVE; relus are split ACT / GPSIMD / DVE.
GPS_RELU = 8     # this many channels run relu on GPSIMD
DVE_RELU = 0     # this many channels run relu on DVE (rest on ACT)


def _schedule(r_ids):
    """Rank-batch relations across cores: sort by count desc, batch j holds
    ranks 8j..8j+7, one relation per core (greedy within batch to balance
    core totals).  Slot capacities (= max count in batch, rounded up to even)
    are shared across cores so the SPMD program is identical everywhere."""
    counts = np.bincount(r_ids, minlength=R_TOT)
    order = np.argsort(-counts, kind="stable")
    G = -(-R_TOT // N_CORES)
    bins = [[None] * G for _ in range(N_CORES)]
    bin_tot = np.zeros(N_CORES, dtype=np.int64)
    caps = []
    for j in range(G):
        batch = [int(r) for r in order[8 * j:8 * j + 8]]
        caps.append(max(2, int(-(-int(counts[batch[0]]) // 2) * 2)))
        # largest counts to least-loaded cores
        ks = list(np.argsort(bin_tot, kind="stable"))
        for i, r in enumerate(batch):
            k = ks[i]
            bins[k][j] = r
            bin_tot[k] += counts[r]
    offs = np.concatenate([[0], np.cumsum(caps)])
    cols = int(offs[-1])
    chunks = []
    o = 0
    n_chunks = -(-cols // PCHUNK)
    base = -(-cols // n_chunks)
    base += base % 2
    while o < cols:
        w = min(base, cols - o)
        chunks.append((o, w))
        o += w
    return bins, G, caps, offs, cols, chunks


def _chan_params(conv_w, conv_b):
    ps = []
    for c in range(C):
        a, w1, e = float(conv_w[c, 0]), float(conv_w[c, 1]), float(conv_w[c, 2])
        main_is_ph = abs(a) >= abs(e)
        s = a if main_is_ph else e
        assert abs(s) > 1e-12, "degenerate conv channel"
        ratio = (e if main_is_ph else a) / s
        sgn = 1.0 if s > 0 else -1.0
        ps.append(dict(
            main_is_ph=main_is_ph, s=s, abs_s=abs(s), sgn=sgn, ratio=ratio,
            wr=w1 / s, cbr=float(conv_b[c]) / s,
        ))
    return ps


def _build_program(G, caps, offs, cols, chunks, chans):
    nc = bacc.Bacc("TRN2", target_bir_lowering=False)
    eht_d = nc.declare_dram_parameter("eht", [D, 2 * cols], F16, isOutput=False)
    wb_d = nc.declare_dram_parameter("wb", [D, G * D], F16, isOutput=False)
    reb_d = nc.declare_dram_parameter("reb", [D, cols], F16, isOutput=False)
    fcs_d = nc.declare_dram_parameter("fcs", [D, C], F16, isOutput=False)
    cbv_d = nc.declare_dram_parameter("cbv", [D, C], F32, isOutput=False)
    out_d = nc.declare_dram_parameter("out", [1, cols], F32, isOutput=True)

    with tile.TileContext(nc) as tc, ExitStack() as ctx:
        cpool = ctx.enter_context(tc.tile_pool(name="const", bufs=1))
        eht = cpool.tile([D, 2 * cols], F16, tag="eht")
        wb = cpool.tile([D, G * D], F16, tag="wb")
        reb = cpool.tile([D, cols], F16, tag="reb")
        fcs = cpool.tile([D, C], F16, tag="fcs")
        cbv = cpool.tile([D, C], F32, tag="cbv")
        ph = cpool.tile([D, cols], F16, tag="ph")
        pt = cpool.tile([D, cols], F16, tag="pt")
        nc.sync.dma_start(out=wb[:], in_=wb_d[:])
        nc.sync.dma_start(out=eht[:], in_=eht_d[:])
        nc.sync.dma_start(out=reb[:], in_=reb_d[:])
        nc.sync.dma_start(out=fcs[:], in_=fcs_d[:])
        nc.sync.dma_start(out=cbv[:], in_=cbv_d[:])

        pproj = ctx.enter_context(tc.tile_pool(name="pproj", bufs=3, space="PSUM"))
        pout = ctx.enter_context(tc.tile_pool(name="pout", bufs=1, space="PSUM"))
        qp = ctx.enter_context(tc.tile_pool(name="qp", bufs=3))
        vp = ctx.enter_context(tc.tile_pool(name="vp", bufs=3))
        rp = ctx.enter_context(tc.tile_pool(name="rp", bufs=3))
        osb = ctx.enter_context(tc.tile_pool(name="osb", bufs=2))

        # ---- projection: per relation group, [PH | PT] = W_g^T @ [Eh | Et]
        for j in range(G):
            cap, off = caps[j], int(offs[j])
            ps = pproj.tile([D, 2 * cap], F32, tag="ps")
            nc.tensor.matmul(
                ps[:], wb[:, j * D:(j + 1) * D],
                eht[:, 2 * off:2 * off + 2 * cap],
            )
            # copy+convert to fp16 (split between ACT and DVE)
            if j % 2 == 0:
                nc.scalar.copy(ph[:, off:off + cap], ps[:, :cap])
                nc.vector.tensor_copy(pt[:, off:off + cap], ps[:, cap:2 * cap])
            else:
                nc.vector.tensor_copy(ph[:, off:off + cap], ps[:, :cap])
                nc.scalar.copy(pt[:, off:off + cap], ps[:, cap:2 * cap])

        # ---- conv + relu + fc, full-width per channel
        po = []
        for i, (o, w) in enumerate(chunks):
            po_i = pout.tile([1, w], F32, tag=f"po{i}", name=f"po{i}")
            po.append(po_i)
        for c in range(C):
            p = chans[c]
            main = ph if p["main_is_ph"] else pt
            oth = pt if p["main_is_ph"] else ph
            q = qp.tile([D, cols], F16, tag="q")
            nc.vector.scalar_tensor_tensor(
                q[:], oth[:], p["ratio"], main[:],
                mybir.AluOpType.mult, mybir.AluOpType.add,
            )
            v = vp.tile([D, cols], F16, tag="v")
            nc.vector.scalar_tensor_tensor(
                v[:], reb[:], p["sgn"] * p["wr"], q[:],
                mybir.AluOpType.mult,
                mybir.AluOpType.add if p["sgn"] > 0 else mybir.AluOpType.subtract,
            )
            r = rp.tile([D, cols], F16, tag="r")
            if c < GPS_RELU:
                nc.gpsimd.tensor_scalar(
                    r[:], v[:], p["sgn"] * p["cbr"], 0.0,
                    mybir.AluOpType.add, mybir.AluOpType.max,
                )
            elif c < GPS_RELU + DVE_RELU:
                nc.vector.tensor_scalar(
                    r[:], v[:], p["sgn"] * p["cbr"], 0.0,
                    mybir.AluOpType.add, mybir.AluOpType.max,
                )
            else:
                nc.scalar.activation(
                    r[:], v[:], mybir.ActivationFunctionType.Relu,
                    bias=cbv[:, c:c + 1], scale=1.0,
                )
            for i, (o, w) in enumerate(chunks):
                nc.tensor.matmul(
                    po[i][:], fcs[:, c:c + 1], r[:, o:o + w],
                    start=(c == 0), stop=(c == C - 1),
                )
        ob = osb.tile([1, cols], F32, tag="ob")
        for i, (o, w) in enumerate(chunks):
            nc.scalar.copy(ob[:, o:o + w], po[i][:])
        nc.sync.dma_start(out=out_d[:], in_=ob[:])
    nc.finalize()
    return nc


def kernel(**inputs):
    triples = np.asarray(inputs["triples"])
    ent_emb = np.asarray(inputs["ent_emb"], dtype=np.float32)
    rel_emb = np.asarray(inputs["rel_emb"], dtype=np.float32)
    rel_W = np.asarray(inputs["rel_W"], dtype=np.float32)
    conv_w = np.asarray(inputs["conv_w"], dtype=np.float32)
    conv_b = np.asarray(inputs["conv_b"], dtype=np.float32)
    fc_w = np.asarray(inputs["fc_w"], dtype=np.float32)
    fc_b = np.asarray(inputs["fc_b"], dtype=np.float32)

    B = triples.shape[0]
    h_ids, r_ids, t_ids = (np.asarray(triples[:, k]).astype(np.int64) for k in range(3))

    bins, G, caps, offs, cols, chunks = _schedule(r_ids)
    chans = _chan_params(conv_w, conv_b)

    fc2 = fc_w.reshape(C, D).astype(np.float32)
    mult = np.array([p["abs_s"] for p in chans], dtype=np.float32)
    fcs_host = np.ascontiguousarray((fc2 * mult[:, None]).T).astype(np.float16)
    cbv_host = np.ascontiguousarray(np.tile(
        np.array([p["sgn"] * p["cbr"] for p in chans], dtype=np.float32)[None, :],
        (D, 1)))

    # triple index lists per relation
    order = np.argsort(r_ids, kind="stable")
    r_sorted = r_ids[order]
    bounds = np.searchsorted(r_sorted, np.arange(R_TOT + 1))
    by_rel = {r: order[bounds[r]:bounds[r + 1]] for r in range(R_TOT)}

    ent16 = ent_emb.astype(np.float16)
    rel16 = rel_emb.astype(np.float16)
    W16 = rel_W.astype(np.float16)

    in_maps = []
    core_meta = []
    for k in range(N_CORES):
        eht_host = np.zeros((D, 2 * cols), dtype=np.float16)
        reb_host = np.zeros((D, cols), dtype=np.float16)
        wb_host = np.zeros((D, G * D), dtype=np.float16)
        meta = []
        for j in range(G):
            cap, off = caps[j], int(offs[j])
            r = bins[k][j]
            if r is None:
                meta.append((j, np.empty(0, dtype=np.int64)))
                continue
            idx = by_rel[r]
            n = len(idx)
            assert n <= cap, (n, cap)
            if n:
                eht_host[:, 2 * off:2 * off + n] = ent16[h_ids[idx]].T
                eht_host[:, 2 * off + cap:2 * off + cap + n] = ent16[t_ids[idx]].T
            reb_host[:, off:off + cap] = rel16[r][:, None]
            wb_host[:, j * D:(j + 1) * D] = W16[r]
            meta.append((j, idx))
        in_maps.append({
            "eht": eht_host, "wb": wb_host, "reb": reb_host, "fcs": fcs_host,
            "cbv": cbv_host,
        })
        core_meta.append(meta)

    nc = _build_program(G, caps, offs, cols, chunks, chans)
    res = run_bass_kernel_spmd(nc, in_maps, list(range(N_CORES)))

    out = np.zeros((B, 1), dtype=np.float32)
    fcb = float(fc_b.reshape(-1)[0])
    for k in range(N_CORES):
        vals = np.asarray(res.results[k]["out"]).reshape(-1)
        for (j, idx) in core_meta[k]:
            if len(idx):
                off = int(offs[j])
                out[idx, 0] = vals[off:off + len(idx)] + fcb
    return out


if __name__ == "__main__":
    rng = np.random.default_rng(0)
    B = 16384
    ins = {
        "triples": rng.integers(0, 237, (B, 3)),
        "ent_emb": rng.standard_normal((50000, D), dtype=np.float32),
        "rel_emb": rng.standard_normal((R_TOT, D), dtype=np.float32),
        "rel_W": rng.standard_normal((R_TOT, D, D), dtype=np.float32) * 0.088,
        "conv_w": rng.standard_normal((C, 3), dtype=np.float32) * 0.1,
        "conv_b": rng.standard_normal((C,), dtype=np.float32) * 0.1,
        "fc_w": rng.standard_normal((1, C * D), dtype=np.float32) * 0.01,
        "fc_b": rng.standard_normal((1,), dtype=np.float32) * 0.01,
    }
    o = kernel(**ins)
    print(o.shape, o[:4, 0])
